# revision 10
# baseline (speedup 1.0000x reference)
"""Segment mean-pooling (scatter_mean) on 8 Trainium2 NeuronCores.

Strategy (segment-sharded, host-routed):
  - The output segment range [0, 100352) is sharded across the 8 cores
    (12544 segments each), so each core produces a disjoint slice of the
    output table and no all-reduce is needed.
  - The host stable-sorts rows by segment id (this is the shard/routing
    step: each row is sent to the core that owns its segment), casts x
    to bf16, appends a ones column (for counts), and packs each core's
    rows into per-bucket tiles of 128 rows. A bucket is an 8-segment
    window; tile counts per bucket position are maxed across cores so
    a single SPMD program serves all 8 cores.
  - Device, per core: stream the packed rows contiguously (no indirect
    DMA), build bf16 one-hots in 32-tile batches from the per-row
    segment-lo values (broadcast tensor_tensor is_equal on the DVE
    engine), and matmul-accumulate [33, 8] blocks into a shared
    [33, 512] PSUM group (64 buckets per PSUM bank). The Act engine
    evacuates finished PSUM groups into an SBUF-resident output table
    [33, 12544] which is written back in one bulk DMA.
  - Host: concatenate the 8 disjoint slices, divide sums by
    max(count, 1), transpose to [100000, 32].
"""
import numpy as np
import ml_dtypes
import concourse.bacc as bacc
import concourse.tile as tile
import concourse.mybir as mybir
from concourse.bass_utils import run_bass_kernel_spmd

F32 = mybir.dt.float32
BF16 = mybir.dt.bfloat16
OP = mybir.AluOpType
ACT_COPY = mybir.ActivationFunctionType.Copy

N_ROWS = 4000000
D = 32
NUM_SEGMENTS = 100000
N_CORES = 8
W = 8                  # segments per bucket (one-hot width)
GB = 64                # buckets per PSUM group ([33, 512] = one 2KB bank)
E = 33                 # packed row: x(32) | 1.0
S_PAD = 100352         # 8 * 12544, >= NUM_SEGMENTS
SEG_PER_CORE = S_PAD // N_CORES      # 12544
NB = SEG_PER_CORE // W               # 196 buckets per core
CHUNK = 256            # tiles per xe load
SUB = 32               # tiles per one-hot batch instruction
DVE_SHARE = 1.0        # fraction of one-hot batches on DVE (rest on Pool;
                       # Pool TensorTensor is not walrus-legal on trn2)

_cache = {}


def _build(tiles):
    """Build the SPMD kernel for per-bucket tile counts `tiles` (len NB,
    every entry >= 1; identical across cores)."""
    total_tiles = sum(tiles)
    R = total_tiles * 128
    nc = bacc.Bacc("TRN2", target_bir_lowering=False, debug=False,
                   num_devices=N_CORES)
    xe_d = nc.dram_tensor("xe", [R * E], BF16, kind="ExternalInput")
    lo_d = nc.dram_tensor("lo", [R], BF16, kind="ExternalInput")
    iota_d = nc.dram_tensor("iota", [128, W], BF16, kind="ExternalInput")
    out_d = nc.dram_tensor("tab", [E, SEG_PER_CORE], F32,
                           kind="ExternalOutput")
    groups = [list(range(g0, min(g0 + GB, NB))) for g0 in range(0, NB, GB)]
    with tile.TileContext(nc) as tc:
        with tc.tile_pool(name="const", bufs=1) as cp, \
             tc.tile_pool(name="stream", bufs=3) as pool, \
             tc.tile_pool(name="ohp", bufs=8) as ohpool, \
             tc.tile_pool(name="psum", bufs=6, space="PSUM") as pp:
            iota = cp.tile([128, W], BF16)
            nc.sync.dma_start(out=iota[:], in_=iota_d.ap())
            ost = cp.tile([E, SEG_PER_CORE], F32)
            g_base = 0     # running tile offset
            acc = 0.0      # DVE/Pool alternation accumulator
            for g, bks in enumerate(groups):
                gw = len(bks) * W
                Tg = sum(tiles[b] for b in bks)
                ps = pp.tile([E, GB * W], F32, space="PSUM", tag="ps")
                xe_g = xe_d.ap()[g_base * 128 * E:(g_base + Tg) * 128 * E] \
                    .rearrange("(p q) -> p q", p=128)
                lo_g = lo_d.ap()[g_base * 128:(g_base + Tg) * 128] \
                    .rearrange("(p q) -> p q", p=128)
                seq = [(b, i) for b in bks for i in range(tiles[b])]
                for c0 in range(0, len(seq), CHUNK):
                    sub = seq[c0:c0 + CHUNK]
                    nsub = len(sub)
                    xe = pool.tile([128, nsub * E], BF16, tag="xe")
                    nc.sync.dma_start(out=xe[:],
                                      in_=xe_g[:, c0 * E:(c0 + nsub) * E])
                    lof = pool.tile([128, nsub], BF16, tag="lo")
                    nc.sync.dma_start(out=lof[:], in_=lo_g[:, c0:c0 + nsub])
                    # one-hots in SUB-tile batches, alternating DVE/Pool
                    for s0 in range(0, nsub, SUB):
                        ns = min(SUB, nsub - s0)
                        oh = ohpool.tile([128, SUB * W], BF16, tag="oh")
                        oh3 = oh[:].rearrange("p (t w) -> p t w", w=W)
                        acc += DVE_SHARE
                        if acc >= 1.0:
                            acc -= 1.0
                            eng = nc.vector
                        else:
                            eng = nc.gpsimd
                        eng.tensor_tensor(
                            out=oh3[:, :ns, :],
                            in0=lof[:, s0:s0 + ns].unsqueeze(-1)
                                .to_broadcast([128, ns, W]),
                            in1=iota[:].unsqueeze(1).to_broadcast([128, ns, W]),
                            op=OP.is_equal)
                        for j in range(s0, s0 + ns):
                            b, i = sub[j]
                            cw = (b - bks[0]) * W
                            nc.tensor.matmul(
                                out=ps[:, cw:cw + W],
                                lhsT=xe[:, j * E:(j + 1) * E],
                                rhs=oh[:, (j - s0) * W:(j - s0 + 1) * W],
                                start=(i == 0), stop=(i == tiles[b] - 1))
                nc.scalar.activation(out=ost[:, g * GB * W:g * GB * W + gw],
                                     in_=ps[:, :gw], func=ACT_COPY)
                g_base += Tg
            nc.sync.dma_start(out=out_d.ap(), in_=ost[:])
    nc.compile()
    return nc


def _pack_core(xb_sorted, lo_sorted, gbkt_sorted, row0, row1, core, tiles):
    """Pack one core's sorted rows into group-major [128, Tg, E] blocks.

    xb_sorted: [N, E] bf16 rows (x | 1), sorted by segment id.
    lo_sorted: [N] bf16 segment-lo (idx % W) per sorted row.
    gbkt_sorted: [N] int32 global bucket id (idx // W) per sorted row.
    Rows [row0, row1) belong to this core.
    """
    xeb = []
    lob = []
    bkt = gbkt_sorted[row0:row1] - core * NB
    # bucket start offsets within the core's row range
    starts = np.searchsorted(bkt, np.arange(NB + 1))
    for b in range(NB):
        Tb = tiles[b]
        a, z = row0 + starts[b], row0 + starts[b + 1]
        nb_rows = z - a
        xx = np.zeros((Tb * 128, E), ml_dtypes.bfloat16)
        xx[:nb_rows] = xb_sorted[a:z]
        ll = np.full(Tb * 128, -1.0, ml_dtypes.bfloat16)
        ll[:nb_rows] = lo_sorted[a:z]
        xeb.append(np.ascontiguousarray(
            xx.reshape(Tb, 128, E).transpose(1, 0, 2)))
        lob.append(np.ascontiguousarray(ll.reshape(Tb, 128).T))
    xe_parts = []
    lo_parts = []
    for g0 in range(0, NB, GB):
        xe_parts.append(np.concatenate(xeb[g0:g0 + GB], axis=1).ravel())
        lo_parts.append(np.concatenate(lob[g0:g0 + GB], axis=1).ravel())
    return np.concatenate(xe_parts), np.concatenate(lo_parts)


def kernel(x, index):
    x = np.asarray(x, dtype=np.float32)
    idx = np.asarray(index).astype(np.int32)
    assert x.shape == (N_ROWS, D) and idx.shape == (N_ROWS,)

    # --- host routing: sort rows by segment, shard segment ranges ---
    order = np.argsort(idx, kind="stable")
    sidx = idx[order]
    gbkt = sidx // W                                   # global bucket id
    cnt = np.bincount(gbkt, minlength=N_CORES * NB).reshape(N_CORES, NB)
    tiles = np.maximum(((cnt + 127) // 128).max(axis=0), 1)  # max over cores
    tiles = [int(t) for t in tiles]

    key = tuple(tiles)
    if _cache.get("key") != key:
        _cache["nc"] = _build(tiles)
        _cache["key"] = key
    nc = _cache["nc"]

    xb = np.zeros((N_ROWS, E), ml_dtypes.bfloat16)
    xb[:, :D] = x[order].astype(ml_dtypes.bfloat16)
    xb[:, D] = 1.0
    lo = (sidx % W).astype(ml_dtypes.bfloat16)
    bounds = np.searchsorted(sidx, np.arange(N_CORES + 1) * SEG_PER_CORE)
    iota = np.tile(np.arange(W, dtype=np.float32), (128, 1)) \
        .astype(ml_dtypes.bfloat16)

    in_maps = []
    for c in range(N_CORES):
        xe_c, lo_c = _pack_core(xb, lo, gbkt, bounds[c], bounds[c + 1],
                                c, tiles)
        in_maps.append({"xe": xe_c, "lo": lo_c, "iota": iota})

    res = run_bass_kernel_spmd(nc, in_maps, list(range(N_CORES))).results

    tab = np.concatenate([res[c]["tab"] for c in range(N_CORES)], axis=1)
    sums = tab[:D, :NUM_SEGMENTS].T.astype(np.float64)
    counts = tab[D, :NUM_SEGMENTS].astype(np.float64)
    out = sums / np.maximum(counts, 1.0)[:, None]
    return out.astype(np.float32)


# revision 11
# speedup vs baseline: 1.0002x; 1.0002x over previous
"""Segment mean-pooling (scatter_mean) on 8 Trainium2 NeuronCores.

Strategy (segment-sharded, host-routed):
  - The output segment range [0, 100352) is sharded across the 8 cores
    (12544 segments each), so each core produces a disjoint slice of the
    output table and no all-reduce is needed.
  - The host stable-sorts rows by segment id (this is the shard/routing
    step: each row is sent to the core that owns its segment), casts x
    to bf16, appends a ones column (for counts), and packs each core's
    rows into per-bucket tiles of 128 rows. A bucket is an 8-segment
    window; tile counts per bucket position are maxed across cores so
    a single SPMD program serves all 8 cores.
  - Device, per core: stream the packed rows contiguously (no indirect
    DMA), build bf16 one-hots in 32-tile batches from the per-row
    segment-lo values (broadcast tensor_tensor is_equal on the DVE
    engine), and matmul-accumulate [33, 8] blocks into a shared
    [33, 512] PSUM group (64 buckets per PSUM bank). The Act engine
    evacuates finished PSUM groups into an SBUF-resident output table
    [33, 12544] which is written back in one bulk DMA.
  - Host: concatenate the 8 disjoint slices, divide sums by
    max(count, 1), transpose to [100000, 32].
"""
import numpy as np
import ml_dtypes
import concourse.bacc as bacc
import concourse.tile as tile
import concourse.mybir as mybir
from concourse.bass_utils import run_bass_kernel_spmd

F32 = mybir.dt.float32
BF16 = mybir.dt.bfloat16
OP = mybir.AluOpType
ACT_COPY = mybir.ActivationFunctionType.Copy

N_ROWS = 4000000
D = 32
NUM_SEGMENTS = 100000
N_CORES = 8
W = 8                  # segments per bucket (one-hot width)
GB = 64                # buckets per PSUM group ([33, 512] = one 2KB bank)
E = 33                 # packed row: x(32) | 1.0
S_PAD = 100352         # 8 * 12544, >= NUM_SEGMENTS
SEG_PER_CORE = S_PAD // N_CORES      # 12544
NB = SEG_PER_CORE // W               # 196 buckets per core
CHUNK = 256            # tiles per xe load
SUB = 32               # tiles per one-hot batch instruction
DVE_SHARE = 1.0        # fraction of one-hot batches on DVE (rest on Pool;
                       # Pool TensorTensor is not walrus-legal on trn2)

_cache = {}


def _build(tiles):
    """Build the SPMD kernel for per-bucket tile counts `tiles` (len NB,
    every entry >= 1; identical across cores)."""
    total_tiles = sum(tiles)
    R = total_tiles * 128
    nc = bacc.Bacc("TRN2", target_bir_lowering=False, debug=False,
                   num_devices=N_CORES)
    xe_d = nc.dram_tensor("xe", [R * E], BF16, kind="ExternalInput")
    lo_d = nc.dram_tensor("lo", [R], BF16, kind="ExternalInput")
    iota_d = nc.dram_tensor("iota", [128, W], BF16, kind="ExternalInput")
    out_d = nc.dram_tensor("tab", [E, SEG_PER_CORE], F32,
                           kind="ExternalOutput")
    groups = [list(range(g0, min(g0 + GB, NB))) for g0 in range(0, NB, GB)]
    with tile.TileContext(nc) as tc:
        with tc.tile_pool(name="const", bufs=1) as cp, \
             tc.tile_pool(name="stream", bufs=3) as pool, \
             tc.tile_pool(name="ohp", bufs=8) as ohpool, \
             tc.tile_pool(name="psum", bufs=6, space="PSUM") as pp:
            iota = cp.tile([128, W], BF16)
            nc.sync.dma_start(out=iota[:], in_=iota_d.ap())
            ost = cp.tile([E, SEG_PER_CORE], F32)
            g_base = 0     # running tile offset
            acc = 0.0      # DVE/Pool alternation accumulator
            for g, bks in enumerate(groups):
                gw = len(bks) * W
                Tg = sum(tiles[b] for b in bks)
                ps = pp.tile([E, GB * W], F32, space="PSUM", tag="ps")
                xe_g = xe_d.ap()[g_base * 128 * E:(g_base + Tg) * 128 * E] \
                    .rearrange("(p q) -> p q", p=128)
                lo_g = lo_d.ap()[g_base * 128:(g_base + Tg) * 128] \
                    .rearrange("(p q) -> p q", p=128)
                seq = [(b, i) for b in bks for i in range(tiles[b])]
                for c0 in range(0, len(seq), CHUNK):
                    sub = seq[c0:c0 + CHUNK]
                    nsub = len(sub)
                    xe = pool.tile([128, nsub * E], BF16, tag="xe")
                    nc.sync.dma_start(out=xe[:],
                                      in_=xe_g[:, c0 * E:(c0 + nsub) * E])
                    lof = pool.tile([128, nsub], BF16, tag="lo")
                    nc.sync.dma_start(out=lof[:], in_=lo_g[:, c0:c0 + nsub])
                    # one-hots in SUB-tile batches, alternating DVE/Pool
                    for s0 in range(0, nsub, SUB):
                        ns = min(SUB, nsub - s0)
                        oh = ohpool.tile([128, SUB * W], BF16, tag="oh")
                        oh3 = oh[:].rearrange("p (t w) -> p t w", w=W)
                        acc += DVE_SHARE
                        if acc >= 1.0:
                            acc -= 1.0
                            eng = nc.vector
                        else:
                            eng = nc.gpsimd
                        eng.tensor_tensor(
                            out=oh3[:, :ns, :],
                            in0=lof[:, s0:s0 + ns].unsqueeze(-1)
                                .to_broadcast([128, ns, W]),
                            in1=iota[:].unsqueeze(1).to_broadcast([128, ns, W]),
                            op=OP.is_equal)
                        for j in range(s0, s0 + ns):
                            b, i = sub[j]
                            cw = (b - bks[0]) * W
                            nc.tensor.matmul(
                                out=ps[:, cw:cw + W],
                                lhsT=xe[:, j * E:(j + 1) * E],
                                rhs=oh[:, (j - s0) * W:(j - s0 + 1) * W],
                                start=(i == 0), stop=(i == tiles[b] - 1))
                nc.scalar.activation(out=ost[:, g * GB * W:g * GB * W + gw],
                                     in_=ps[:, :gw], func=ACT_COPY)
                g_base += Tg
            nc.sync.dma_start(out=out_d.ap(), in_=ost[:])
    nc.compile()
    return nc


def _pack_core(xb_sorted, lo_sorted, gbkt_sorted, row0, row1, core, tiles,
               order):
    """Pack one core's sorted rows into group-major [128, Tg, E] blocks.

    xb_sorted: [N, E] bf16 rows (x | 1), sorted by segment id.
    lo_sorted: [N] bf16 segment-lo (idx % W) per sorted row.
    gbkt_sorted: [N] int32 global bucket id (idx // W) per sorted row.
    Rows [row0, row1) belong to this core.
    """
    xeb = []
    lob = []
    bkt = gbkt_sorted[row0:row1] - core * NB
    # bucket start offsets within the core's row range
    starts = np.searchsorted(bkt, np.arange(NB + 1))
    for k in range(NB):
        b = order[k]          # this core's bucket assigned to program slot k
        Tb = tiles[k]
        a, z = row0 + starts[b], row0 + starts[b + 1]
        nb_rows = z - a
        xx = np.zeros((Tb * 128, E), ml_dtypes.bfloat16)
        xx[:nb_rows] = xb_sorted[a:z]
        ll = np.full(Tb * 128, -1.0, ml_dtypes.bfloat16)
        ll[:nb_rows] = lo_sorted[a:z]
        xeb.append(np.ascontiguousarray(
            xx.reshape(Tb, 128, E).transpose(1, 0, 2)))
        lob.append(np.ascontiguousarray(ll.reshape(Tb, 128).T))
    xe_parts = []
    lo_parts = []
    for g0 in range(0, NB, GB):
        xe_parts.append(np.concatenate(xeb[g0:g0 + GB], axis=1).ravel())
        lo_parts.append(np.concatenate(lob[g0:g0 + GB], axis=1).ravel())
    return np.concatenate(xe_parts), np.concatenate(lo_parts)


def kernel(x, index):
    x = np.asarray(x, dtype=np.float32)
    idx = np.asarray(index).astype(np.int32)
    assert x.shape == (N_ROWS, D) and idx.shape == (N_ROWS,)

    # --- host routing: sort rows by segment, shard segment ranges ---
    order = np.argsort(idx, kind="stable")
    sidx = idx[order]
    gbkt = sidx // W                                   # global bucket id
    cnt = np.bincount(gbkt, minlength=N_CORES * NB).reshape(N_CORES, NB)
    ceil = np.maximum((cnt + 127) // 128, 1)
    # Slot-matched tiling: each core maps its k-th fullest bucket to
    # program slot k, so slot capacity = max of the cores' k-th largest
    # counts (far tighter than a positional max). The host unpermutes
    # the output columns per core afterwards.
    order_all = np.argsort(-ceil, axis=1, kind="stable")      # [cores, NB]
    tiles = [int(t) for t in
             np.take_along_axis(ceil, order_all, axis=1).max(axis=0)]

    key = tuple(tiles)
    if _cache.get("key") != key:
        _cache["nc"] = _build(tiles)
        _cache["key"] = key
    nc = _cache["nc"]

    xb = np.zeros((N_ROWS, E), ml_dtypes.bfloat16)
    xb[:, :D] = x[order].astype(ml_dtypes.bfloat16)
    xb[:, D] = 1.0
    lo = (sidx % W).astype(ml_dtypes.bfloat16)
    bounds = np.searchsorted(sidx, np.arange(N_CORES + 1) * SEG_PER_CORE)
    iota = np.tile(np.arange(W, dtype=np.float32), (128, 1)) \
        .astype(ml_dtypes.bfloat16)

    in_maps = []
    for c in range(N_CORES):
        xe_c, lo_c = _pack_core(xb, lo, gbkt, bounds[c], bounds[c + 1],
                                c, tiles, order_all[c])
        in_maps.append({"xe": xe_c, "lo": lo_c, "iota": iota})

    res = run_bass_kernel_spmd(nc, in_maps, list(range(N_CORES))).results

    slices = []
    for c in range(N_CORES):
        slot = res[c]["tab"].reshape(E, NB, W)
        byb = np.empty_like(slot)
        byb[:, order_all[c], :] = slot        # slot k held bucket order[k]
        slices.append(byb.reshape(E, NB * W))
    tab = np.concatenate(slices, axis=1)
    sums = tab[:D, :NUM_SEGMENTS].T.astype(np.float64)
    counts = tab[D, :NUM_SEGMENTS].astype(np.float64)
    out = sums / np.maximum(counts, 1.0)[:, None]
    return out.astype(np.float32)
